# revision 53
# baseline (speedup 1.0000x reference)
"""Trainium2 Bass kernel for the CSMAdapter module.

Contract: kernel(**inputs) takes the FULL unsharded inputs (as produced by
the reference setup_inputs()) and returns the FULL output [4, 100, 1024].

Strategy
--------
All weight-only computation is folded on the host (it is data-independent):
    w_proj   = W_in @ Wd.T + bd
    w_prime  = P.T @ w_proj @ P
    masked_w = w_prime * sigmoid(spectral_mask)
    A        = P @ masked_w.T @ P.T          # fused = x @ A
    W_big    = W_in.T @ A                    # fused = llama @ W_big + b_in @ A
The final LayerNorm + mel projection algebra is folded into the mel GEMM:
    mel[m,t] = rstd[t]*(Wg @ h2)[m,t] - (mu[t]*rstd[t])*c1[m] + c2[m]
with Wg = Wmel * ln_g, c1 = Wmel @ ln_g, c2 = Wmel @ ln_b + bmel.
The channel mean is folded into the mel GEMM as an extra ones column at
lhsT position 96 (PSUM partition reads must be 32-aligned); the four mel
rows it displaces live at positions 97..100 and the output is DMA'd out
as two partition ranges.

Device (SPMD over 8 cores, data-parallel over the 4096 tokens, 512 each +
2-token conv halos), all heavy matmuls in bf16 (fp32 PSUM accumulation —
same 1 col/cycle PE rate as fp32r but half the HBM traffic):
  Phase A: big GEMM k-major over d-tiles 0-3 simultaneously (4 PSUM banks)
           so the PE stays saturated while the x k-tiles stream from HBM
           (one arriving k-tile unlocks 4 matmuls).
  Phase B: d-major big GEMM for d-tiles 4-7 (x now SBUF-resident),
           interleaved with conv1 -> gelu -> conv2 (block-diagonal per-tap
           matmuls, groups=16) and the mel/stats GEMMs for finished tiles.
  Tail:    LayerNorm stats chain on [1,512], rank-1 correction matmuls,
           combine, output DMA.
"""

import sys

import numpy as np


def _ensure_concourse():
    try:
        import concourse  # noqa: F401
    except ImportError:  # pragma: no cover
        for p in ("/opt/trn_rl_repo", "/root/.axon_site/_ro/trn_rl_repo"):
            if p not in sys.path:
                sys.path.insert(0, p)


# ---- static shapes ----
B, T, L, D = 4, 1024, 3072, 1024
NCORES = 8
TOK = 512            # owned tokens per core
EXT = TOK + 4        # fused ext window: tokens -2 .. TOK+2
G1E = TOK + 2        # conv1 ext output: tokens -1 .. TOK+1
KT = L // 128        # 24
KH = KT // 2         # 12
DT = D // 128        # 8
DA = 4               # d-tiles done k-major in phase A
NMEL = 100
NM1 = NMEL + 1       # mel lhsT cols: 96 rows | ones | 4 displaced rows
GS = 64              # group size (1024 / 16 groups)
GROUPS_ = 16

OFF_C1 = 0
OFF_C2 = OFF_C1 + NM1
OFF_ONES = OFF_C2 + NM1
OFF_TW = OFF_ONES + TOK      # 2.0s: ps_s lhsT (rstd = 2 * Dsqrt output)
SM_LEN = OFF_TW + NM1

LN_EPS = 1e-5
NWU = 72             # PE warmup matmuls: bridge the DMA ramp (~8us) + data
                     # wait (~15us) so phase A starts with HAM at 2.4 GHz

_PROGRAM = None          # cached (nc, input names)
LAST_RESULTS = None      # BassKernelResults of the most recent run (for test.py)


def _build_program():
    _ensure_concourse()
    from concourse import bacc, tile
    import concourse.mybir as mybir

    f32 = mybir.dt.float32
    f32r = mybir.dt.float32r
    bf16 = mybir.dt.bfloat16
    AF = mybir.ActivationFunctionType
    MUL = mybir.AluOpType.mult
    ADD = mybir.AluOpType.add

    nc = bacc.Bacc("TRN2", debug=False, target_bir_lowering=False)

    # DRAM layouts are partition-major so every DMA is contiguous.
    xt_d = nc.dram_tensor("xt", [KT, 128, EXT], bf16, kind="ExternalInput")
    wbig_d = nc.dram_tensor("wbig", [DT * 2, 128, KH, 128], bf16,
                            kind="ExternalInput")
    cw1_d = nc.dram_tensor("cw1", [128, DT, 3, 128], bf16, kind="ExternalInput")
    cw2_d = nc.dram_tensor("cw2", [128, DT, 3, 128], bf16, kind="ExternalInput")
    wgt_d = nc.dram_tensor("wgt", [128, DT, NM1], bf16, kind="ExternalInput")
    cb_d = nc.dram_tensor("cb", [128, 36], f32, kind="ExternalInput")
    sm_d = nc.dram_tensor("smalls", [1, SM_LEN], f32r, kind="ExternalInput")
    onec_d = nc.dram_tensor("onec", [128, 1], bf16, kind="ExternalInput")
    # host-computed halo columns: per d-tile, 4 fused halo cols + 2 g halo cols
    halo_d = nc.dram_tensor("halo", [128, DT, 6], bf16, kind="ExternalInput")
    mel_d = nc.dram_tensor("mel", [NMEL, TOK], f32, kind="ExternalOutput")

    with tile.TileContext(nc) as tc:
        with (
            tc.tile_pool(name="consts", bufs=1) as consts,
            tc.tile_pool(name="wpool", bufs=16) as wpool,
            tc.tile_pool(name="acts", bufs=1) as acts,
            tc.tile_pool(name="stats", bufs=1) as stats,
            tc.tile_pool(name="ps_ga", bufs=4, space="PSUM") as ps_ga,
            tc.tile_pool(name="ps_cv", bufs=2, space="PSUM") as ps_cv,
            tc.tile_pool(name="ps_sq", bufs=1, space="PSUM") as ps_sq_p,
            tc.tile_pool(name="ps_mel", bufs=1, space="PSUM") as ps_mel,
        ):
            wu_sb = consts.tile([128, 128], bf16, name="wu_sb")
            nc.vector.memset(wu_sb, 0.0)
            # ---- PE warmup while the first DMAs stream ----
            ps_wu = ps_cv.tile([128, 128], f32, name="ps_wu", tag="cv")
            for i in range(NWU):
                nc.tensor.matmul(
                    ps_wu, lhsT=wu_sb, rhs=wu_sb,
                    start=(i == 0), stop=(i == NWU - 1),
                )

            wbh = {}

            def load_wbh(i):
                t = wpool.tile([128, KH, 128], bf16, name=f"wbh{i}", tag="wb")
                nc.sync.dma_start(out=t, in_=wbig_d[i])
                wbh[i] = t

            xk = []

            def load_xk(k):
                t = consts.tile([128, EXT], bf16, name=f"xk{k}", tag=f"xk{k}")
                nc.sync.dma_start(out=t, in_=xt_d[k])
                xk.append(t)

            load_wbh(0)
            load_xk(0)
            load_wbh(2)
            load_xk(1)
            load_wbh(4)
            load_xk(2)
            load_wbh(6)
            load_xk(3)
            sm_sb = consts.tile([1, SM_LEN], f32r, name="sm_sb")
            nc.sync.dma_start(out=sm_sb, in_=sm_d[:])
            cb_sb = consts.tile([128, 36], f32, name="cb_sb")
            nc.sync.dma_start(out=cb_sb, in_=cb_d[:])
            ones_col = consts.tile([128, 1], bf16, name="ones_col")
            nc.sync.dma_start(out=ones_col, in_=onec_d[:])
            halo_sb = consts.tile([128, DT, 6], bf16, name="halo_sb")
            nc.sync.dma_start(out=halo_sb, in_=halo_d[:])
            ones_row = sm_sb[0:1, OFF_ONES : OFF_ONES + TOK]
            for k in range(4, 12):
                load_xk(k)
            load_wbh(1)
            load_wbh(3)
            for k in range(12, 16):
                load_xk(k)
            load_wbh(5)
            load_wbh(7)
            load_xk(16)
            load_xk(17)
            load_wbh(8)
            for k in range(18, 21):
                load_xk(k)
            load_wbh(9)
            for k in range(21, 24):
                load_xk(k)
            cw1_sb = consts.tile([128, DT, 3, 128], bf16, name="cw1_sb")
            cw2_sb = consts.tile([128, DT, 3, 128], bf16, name="cw2_sb")
            wgt_sb = consts.tile([128, DT, NM1], bf16, name="wgt_sb")
            nc.scalar.dma_start(out=cw1_sb, in_=cw1_d[:])
            nc.scalar.dma_start(out=cw2_sb, in_=cw2_d[:])
            nc.scalar.dma_start(out=wgt_sb, in_=wgt_d[:])
            for i in range(10, 16):
                load_wbh(i)

            fused = [None] * DT
            g = [None] * DT
            h2 = [None] * DT
            h2sq = [None] * DT
            ps_sq_ref = [None]
            ps_m_ref = [None]

            def fused_copy(d, ps):
                fu = acts.tile([128, EXT], bf16, name=f"fu{d}", tag=f"fu{d}")
                fused[d] = fu
                # bias add + f32->bf16 cast in one scalar op
                nc.scalar.add(out=fu[:, 2 : 2 + TOK], in_=ps,
                              add=cb_sb[:, 27 + d : 28 + d])
                nc.vector.tensor_copy(fu[:, 0:2], halo_sb[:, d, 0:2])
                nc.vector.tensor_copy(fu[:, EXT - 2 : EXT], halo_sb[:, d, 2:4])

            # ---- phase A: k-major big GEMM for d-tiles 0..3 ----
            psA = [ps_ga.tile([128, TOK], f32, name=f"psA{d}", tag="ga")
                   for d in range(DA)]
            for k in range(KT):
                for d in range(DA):
                    nc.tensor.matmul(
                        psA[d],
                        lhsT=wbh[2 * d + k // KH][:, k % KH, :],
                        rhs=xk[k][:, 2 : 2 + TOK],
                        start=(k == 0), stop=(k == KT - 1),
                    )
                    if k == KT - 1:
                        # drain each bank as soon as its group stops so the
                        # phase-B gemm can reuse it without a scalar-op stall
                        fused_copy(d, psA[d])
                if k < 12:
                    # filler matmuls: while the x stream still trickles in,
                    # keep the PE busy through data-wait gaps so HAM doesn't
                    # re-throttle the clock to 1.2 GHz (~50ns each when warm)
                    for _ in range(6 if k < 4 else 3):
                        nc.tensor.matmul(ps_wu, lhsT=wu_sb, rhs=wu_sb,
                                         start=True, stop=True)

            # ---- phase B/C building blocks ----
            def gemm(d):
                ps = ps_ga.tile([128, TOK], f32, name=f"psA{d}", tag="ga")
                for k in range(KT):
                    nc.tensor.matmul(
                        ps,
                        lhsT=wbh[2 * d + k // KH][:, k % KH, :],
                        rhs=xk[k][:, 2 : 2 + TOK],
                        start=(k == 0), stop=(k == KT - 1),
                    )
                fused_copy(d, ps)

            def conv1(d):
                # device computes g_ext cols [1, 513); cols 0 and 513 from host
                gd = acts.tile([128, G1E], bf16, name=f"g{d}", tag=f"g{d}")
                g[d] = gd
                ps = ps_cv.tile([128, TOK], f32, name=f"psB{d}", tag="cv")
                for tap in range(3):
                    nc.tensor.matmul(
                        ps, lhsT=cw1_sb[:, d, tap, :],
                        rhs=fused[d][:, 1 + tap : 1 + tap + TOK],
                        start=(tap == 0), stop=(tap == 2),
                    )
                # gelu(conv1 + b1) in one ACT-engine op (erf-based LUT)
                nc.scalar.activation(
                    out=gd[:, 1 : 1 + TOK], in_=ps, func=AF.Gelu,
                    bias=cb_sb[:, 19 + d : 20 + d], scale=1.0,
                )
                nc.vector.tensor_copy(gd[:, 0:1], halo_sb[:, d, 4:5])
                nc.vector.tensor_copy(gd[:, G1E - 1 : G1E], halo_sb[:, d, 5:6])

            def conv2(d):
                h2d = acts.tile([128, TOK], bf16, name=f"h2{d}", tag=f"h2{d}")
                h2sqd = acts.tile([128, TOK], bf16, name=f"h2sq{d}", tag="h2sq",
                                  bufs=2)
                h2[d] = h2d
                h2sq[d] = h2sqd
                ps = ps_cv.tile([128, TOK], f32, name=f"psC{d}", tag="cv")
                for tap in range(3):
                    nc.tensor.matmul(
                        ps, lhsT=cw2_sb[:, d, tap, :],
                        rhs=g[d][:, tap : tap + TOK],
                        start=(tap == 0), stop=(tap == 2),
                    )
                nc.scalar.add(out=h2d, in_=ps, add=cb_sb[:, 8 + d : 9 + d])
                # square on DVE (bf16 in/out, 2x rate) — scalar engine is the
                # phase-C pipeline bottleneck otherwise
                nc.vector.tensor_mul(h2sqd, h2d, h2d)

            def statmm(d):
                if d == 0:
                    ps_sq_ref[0] = ps_sq_p.tile([1, TOK], f32, name="ps_sq",
                                                tag="sq")
                    ps_m_ref[0] = ps_mel.tile([NM1, TOK], f32, name="ps_m",
                                              tag="mel")
                last = d == DT - 1
                nc.tensor.matmul(ps_sq_ref[0], lhsT=ones_col, rhs=h2sq[d][:],
                                 start=(d == 0), stop=last)
                # rows 0..95: Wg rows 0..95; row 96: channel sum (mean);
                # rows 97..100: Wg rows 96..99
                nc.tensor.matmul(ps_m_ref[0], lhsT=wgt_sb[:, d, :], rhs=h2[d][:],
                                 start=(d == 0), stop=last)

            # ---- phase B: d-major gemms 4..7 + pipelined conv/stats ----
            for _ in range(8):
                # bridge the wbh8/9 DMA wait at the A->B transition
                nc.tensor.matmul(ps_wu, lhsT=wu_sb, rhs=wu_sb,
                                 start=True, stop=True)
            gemm(4)
            conv1(0)
            gemm(5)
            conv1(1)
            conv2(0)
            gemm(6)
            conv1(2)
            conv2(1)
            statmm(0)
            gemm(7)
            conv1(3)
            conv2(2)
            statmm(1)
            # ---- phase C: remaining conv/stats ----
            for d in range(4, DT):
                conv1(d)
                conv2(d - 1)
                statmm(d - 2)
            conv2(DT - 1)
            statmm(DT - 2)
            statmm(DT - 1)

            # ---- stats on [1, TOK] ----
            # the mel lhsT's fold column is prescaled to 1/D (exact in bf16)
            # so ps_m row 96 IS the mean; the sq ones-column is prescaled
            # likewise so ps_sq is E[h^2].
            ps_sq = ps_sq_ref[0]
            ps_m = ps_m_ref[0]
            msq = stats.tile([1, TOK], f32, name="msq", tag="sv", bufs=3)
            nc.scalar.activation(msq, ps_m[96:97, :], AF.Square)
            var = stats.tile([1, TOK], f32, name="var", tag="sv", bufs=3)
            nc.vector.scalar_tensor_tensor(
                var, in0=msq, scalar=-1.0, in1=ps_sq, op0=MUL, op1=ADD,
            )
            sqv = stats.tile([1, TOK], f32, name="sqv", tag="sv", bufs=3)
            nc.scalar.activation(sqv, var, AF.Sqrt,
                                 bias=cb_sb[0:1, 18:19], scale=1.0)
            rstd32 = stats.tile([1, TOK], f32, name="rstd32")
            nc.vector.reciprocal_approx_fast(rstd32, sqv)
            rstd = stats.tile([1, TOK], f32r, name="rstd")
            nc.vector.tensor_copy(rstd, rstd32)
            negu = stats.tile([1, TOK], f32r, name="negu")
            nc.vector.scalar_tensor_tensor(
                negu, in0=ps_m[96:97, :], scalar=-1.0, in1=rstd32,
                op0=MUL, op1=MUL,
            )
            # drain the mel PSUM to SBUF while the correction matmuls run
            m_sb = stats.tile([NM1, TOK], f32, name="m_sb")
            nc.vector.tensor_copy(m_sb, ps_m)

            # ---- rank-1 corrections + output ----
            ps_r = ps_cv.tile([NM1, TOK], f32, name="ps_r", tag="cv")
            nc.tensor.matmul(
                ps_r, lhsT=sm_sb[0:1, OFF_C2 : OFF_C2 + NM1],
                rhs=ones_row, start=True, stop=False,
            )
            nc.tensor.matmul(
                ps_r, lhsT=sm_sb[0:1, OFF_C1 : OFF_C1 + NM1],
                rhs=negu[0:1, :], start=False, stop=True,
            )
            ps_s = ps_cv.tile([NM1, TOK], f32, name="ps_s", tag="cv")
            nc.tensor.matmul(
                ps_s, lhsT=sm_sb[0:1, OFF_ONES : OFF_ONES + NM1],
                rhs=rstd[0:1, :], start=True, stop=True,
            )
            out_sb = stats.tile([NM1, TOK], f32, name="out_sb")
            # combine + store in token halves so the first DMA overlaps the
            # second half's DVE work
            HT = TOK // 2
            for c0 in (0, HT):
                sl = slice(c0, c0 + HT)
                nc.vector.tensor_mul(out_sb[:, sl], m_sb[:, sl], ps_s[:, sl])
                nc.vector.tensor_add(out_sb[:, sl], out_sb[:, sl],
                                     ps_r[:, sl])
                nc.sync.dma_start(out=mel_d[0:96, sl], in_=out_sb[0:96, sl])
                nc.sync.dma_start(out=mel_d[96:NMEL, sl],
                                  in_=out_sb[97:NM1, sl])

    nc.compile()
    return nc


def _sigmoid64(x):
    return 1.0 / (1.0 + np.exp(-x.astype(np.float64)))


def _melperm(v):
    """Permute a [100]-vector into the 101-slot layout (slot 96 = 0)."""
    out = np.zeros(NM1, dtype=v.dtype)
    out[0:96] = v[0:96]
    out[97:NM1] = v[96:NMEL]
    return out


def host_prep(inputs):
    """Fold all data-independent computation; build per-core device inputs.

    Returns (shared, per_core) where shared is a dict of replicated arrays
    and per_core is a list of 8 dicts with the core-specific arrays.
    """
    import ml_dtypes

    f32 = np.float32
    bf16 = ml_dtypes.bfloat16
    W_in = np.asarray(inputs["W_in"], dtype=np.float64)
    Wd = np.asarray(inputs["Wd"], dtype=np.float64)
    bd = np.asarray(inputs["bd"], dtype=np.float64)
    P = np.asarray(inputs["P"], dtype=np.float64)
    smask = np.asarray(inputs["spectral_mask"], dtype=np.float64)
    b_in = np.asarray(inputs["b_in"], dtype=np.float64)

    w_proj = W_in @ Wd.T + bd[None, :]
    w_prime = P.T @ w_proj @ P
    masked_w = w_prime * _sigmoid64(smask)
    A = P @ masked_w.T @ P.T
    W_big64 = W_in.T @ A                                       # [L, D] f64
    b_big64 = b_in @ A                                         # [D] f64
    W_big = np.ascontiguousarray(W_big64, dtype=f32)

    # [2d+half, kp, k_in_half, dc] (partition-major, half k-slices)
    wbig_t = np.ascontiguousarray(
        W_big.reshape(2, KH, 128, DT, 128).transpose(3, 0, 2, 1, 4)
    ).reshape(DT * 2, 128, KH, 128).astype(bf16)

    def blockdiag(w):
        w = np.asarray(w, dtype=f32)  # [C, GS, 3]
        out = np.zeros((DT, 3, 128, 128), dtype=f32)
        for d in range(DT):
            for co in range(128):
                c = d * 128 + co
                blk = co // GS
                # out[d, tap, blk*GS + i, co] = w[c, i, tap]
                out[d, :, blk * GS : (blk + 1) * GS, co] = w[c].T
        return out

    cw1_t = np.ascontiguousarray(
        blockdiag(inputs["conv1_w"]).transpose(2, 0, 1, 3)).astype(bf16)
    cw2_t = np.ascontiguousarray(
        blockdiag(inputs["conv2_w"]).transpose(2, 0, 1, 3)).astype(bf16)

    Wmel = np.asarray(inputs["Wmel"], dtype=np.float64)
    ln_g = np.asarray(inputs["ln_g"], dtype=np.float64)
    ln_b = np.asarray(inputs["ln_b"], dtype=np.float64)
    bmel = np.asarray(inputs["bmel"], dtype=np.float64)
    Wg = (Wmel * ln_g[None, :]).astype(f32)                    # [NMEL, D]
    # lhsT col j: j<96 -> Wg row j; 96 -> ones (mean fold); 97..100 -> rows 96..99
    wgt_t = np.zeros((128, DT, NM1), dtype=f32)
    wgt_full = Wg.T.reshape(DT, 128, NMEL).transpose(1, 0, 2)  # [p, d, m]
    wgt_t[:, :, 0:96] = wgt_full[:, :, 0:96]
    wgt_t[:, :, 96] = 1.0 / D                    # mean fold (2^-10, exact)
    wgt_t[:, :, 97:NM1] = wgt_full[:, :, 96:NMEL]
    wgt_t = wgt_t.astype(bf16)
    c1 = _melperm((Wmel @ ln_g).astype(f32))
    c2 = _melperm((Wmel @ ln_b + bmel).astype(f32))

    cb_base = np.zeros((128, 36), dtype=f32)
    cb_base[:, 18] = LN_EPS
    b1_cols = np.asarray(inputs["conv1_b"], dtype=f32).reshape(DT, 128).T
    cb_base[:, 8:16] = np.asarray(inputs["conv2_b"], dtype=f32).reshape(DT, 128).T
    cb_base[:, 19:27] = b1_cols
    cb_base[:, 27:35] = b_big64.astype(f32).reshape(DT, 128).T

    llama = np.asarray(inputs["llama_embeddings"], dtype=f32).reshape(B * T, L)
    conv1_w_np = np.asarray(inputs["conv1_w"], dtype=np.float64)  # [D, GS, 3]
    conv1_b_np = np.asarray(inputs["conv1_b"], dtype=np.float64)
    gidx = np.arange(D) // GS

    import math
    _erf_vec = np.vectorize(math.erf)

    def _gelu64(x):
        return x * 0.5 * (1.0 + _erf_vec(x / math.sqrt(2.0)))

    shared = dict(wbig=wbig_t, cw1=cw1_t, cw2=cw2_t, wgt=wgt_t,
                  onec=np.full((128, 1), 1.0 / D, dtype=bf16))
    per_core = []
    for c in range(NCORES):
        b, h = divmod(c, 2)
        start = b * T + h * TOK
        ext_idx = np.arange(start - 2, start + TOK + 2)
        valid = (ext_idx >= b * T) & (ext_idx < (b + 1) * T)
        xext = np.zeros((EXT, L), dtype=f32)
        xext[valid] = llama[ext_idx[valid]]
        xt = np.ascontiguousarray(
            xext.T.reshape(KT, 128, EXT)
        ).astype(bf16)  # [k, p, t]

        # host-computed halo columns (exact fp32-grade)
        def fcol(u):
            gu = start + u
            if b * T <= gu < (b + 1) * T:
                return llama[gu].astype(np.float64) @ W_big64 + b_big64
            return np.zeros(D, dtype=np.float64)

        def conv1col(m3):
            # m3: [D, 3] inputs for taps 0..2 -> conv1 + bias, gelu
            in_g = m3.reshape(GROUPS_, GS, 3)[gidx]       # [D, GS, 3]
            out = np.einsum("cit,cit->c", conv1_w_np, in_g) + conv1_b_np
            return _gelu64(out)

        fm2, fm1, f0 = fcol(-2), fcol(-1), fcol(0)
        f510, f511 = fcol(510), fcol(511)
        f512, f513 = fcol(TOK), fcol(TOK + 1)
        if h == 1:
            g_left = conv1col(np.stack([fm2, fm1, f0], axis=1))
        else:
            g_left = np.zeros(D, dtype=np.float64)
        if h == 0:
            g_right = conv1col(np.stack([f511, f512, f513], axis=1))
        else:
            g_right = np.zeros(D, dtype=np.float64)
        halo = np.zeros((128, DT, 6), dtype=f32)
        for dd in range(DT):
            slc = slice(dd * 128, (dd + 1) * 128)
            halo[:, dd, 0] = fm2[slc]
            halo[:, dd, 1] = fm1[slc]
            halo[:, dd, 2] = f512[slc]
            halo[:, dd, 3] = f513[slc]
            halo[:, dd, 4] = g_left[slc]
            halo[:, dd, 5] = g_right[slc]

        sm = np.zeros((1, SM_LEN), dtype=f32)
        sm[0, OFF_C1 : OFF_C1 + NM1] = c1
        sm[0, OFF_C2 : OFF_C2 + NM1] = c2
        sm[0, OFF_ONES : OFF_ONES + TOK] = 1.0
        sm[0, OFF_TW : OFF_TW + NM1] = 2.0

        per_core.append(dict(xt=xt, smalls=sm, cb=cb_base,
                             halo=halo.astype(bf16)))
    return shared, per_core


def _ensure_axon_hooks():
    """If this image's antenv lacks axon_hooks (needed by bass_utils when
    BASS_TRACE is set under axon), register a functional ctypes-based hook so
    tracing degrades gracefully instead of crashing."""
    try:
        import antenv.axon_hooks  # noqa: F401
        return
    except ImportError:
        pass
    try:
        import contextlib
        import ctypes
        import types

        hook = None
        try:
            lib = ctypes.CDLL("/opt/axon/libaxon_pjrt.so")
            if hasattr(lib, "axon_start_nrt_profile"):
                lib.axon_start_nrt_profile.argtypes = [
                    ctypes.POINTER(ctypes.c_int64),
                    ctypes.c_size_t,
                ]
                lib.axon_start_nrt_profile.restype = ctypes.c_int64
                lib.axon_stop_nrt_profile.argtypes = [ctypes.c_char_p]
                lib.axon_stop_nrt_profile.restype = ctypes.c_int64

                @contextlib.contextmanager
                def hook(output_dir, device_ids):
                    import jax

                    jax.devices()
                    if device_ids:
                        ids = (ctypes.c_int64 * len(device_ids))(*device_ids)
                        rc = lib.axon_start_nrt_profile(ids, len(device_ids))
                    else:
                        rc = lib.axon_start_nrt_profile(None, 0)
                    if rc != 0:
                        raise RuntimeError(f"axon_start_nrt_profile rc={rc}")
                    try:
                        yield
                    finally:
                        lib.axon_stop_nrt_profile(str(output_dir).encode())
        except OSError:
            hook = None

        mod = types.ModuleType("antenv.axon_hooks")
        mod.get_axon_ntff_profile_hook = lambda: hook
        mod.set_axon_ntff_profile_hook = lambda h: None
        sys.modules["antenv.axon_hooks"] = mod
        import antenv

        antenv.axon_hooks = mod
    except Exception:
        pass


def kernel(**inputs):
    global _PROGRAM, LAST_RESULTS
    _ensure_concourse()
    _ensure_axon_hooks()
    from concourse import bass_utils

    if _PROGRAM is None:
        _PROGRAM = _build_program()
    nc = _PROGRAM

    shared, per_core = host_prep(inputs)
    in_maps = [{**shared, **pc} for pc in per_core]

    res = None
    last_exc = None
    for _attempt in range(3):
        try:
            res = bass_utils.run_bass_kernel_spmd(
                nc, in_maps, core_ids=list(range(NCORES))
            )
            break
        except Exception as exc:  # transient NRT device errors happen
            last_exc = exc
    if res is None:
        raise last_exc
    LAST_RESULTS = res

    out = np.zeros((B, NMEL, T), dtype=np.float32)
    for c in range(NCORES):
        b, h = divmod(c, 2)
        out[b, :, h * TOK : (h + 1) * TOK] = res.results[c]["mel"]
    return out


# revision 55
# speedup vs baseline: 1.0142x; 1.0142x over previous
"""Trainium2 Bass kernel for the CSMAdapter module.

Contract: kernel(**inputs) takes the FULL unsharded inputs (as produced by
the reference setup_inputs()) and returns the FULL output [4, 100, 1024].

Strategy
--------
All weight-only computation is folded on the host (it is data-independent):
    w_proj   = W_in @ Wd.T + bd
    w_prime  = P.T @ w_proj @ P
    masked_w = w_prime * sigmoid(spectral_mask)
    A        = P @ masked_w.T @ P.T          # fused = x @ A
    W_big    = W_in.T @ A                    # fused = llama @ W_big + b_in @ A
The final LayerNorm + mel projection algebra is folded into the mel GEMM:
    mel[m,t] = rstd[t]*(Wg @ h2)[m,t] - (mu[t]*rstd[t])*c1[m] + c2[m]
with Wg = Wmel * ln_g, c1 = Wmel @ ln_g, c2 = Wmel @ ln_b + bmel.
The channel mean is folded into the mel GEMM as an extra ones column at
lhsT position 96 (PSUM partition reads must be 32-aligned); the four mel
rows it displaces live at positions 97..100 and the output is DMA'd out
as two partition ranges.

Device (SPMD over 8 cores, data-parallel over the 4096 tokens, 512 each +
2-token conv halos), all heavy matmuls in bf16 (fp32 PSUM accumulation —
same 1 col/cycle PE rate as fp32r but half the HBM traffic):
  Phase A: big GEMM k-major over d-tiles 0-3 simultaneously (4 PSUM banks)
           so the PE stays saturated while the x k-tiles stream from HBM
           (one arriving k-tile unlocks 4 matmuls).
  Phase B: d-major big GEMM for d-tiles 4-7 (x now SBUF-resident),
           interleaved with conv1 -> gelu -> conv2 (block-diagonal per-tap
           matmuls, groups=16) and the mel/stats GEMMs for finished tiles.
  Tail:    LayerNorm stats chain on [1,512], rank-1 correction matmuls,
           combine, output DMA.
"""

import sys

import numpy as np


def _ensure_concourse():
    try:
        import concourse  # noqa: F401
    except ImportError:  # pragma: no cover
        for p in ("/opt/trn_rl_repo", "/root/.axon_site/_ro/trn_rl_repo"):
            if p not in sys.path:
                sys.path.insert(0, p)


# ---- static shapes ----
B, T, L, D = 4, 1024, 3072, 1024
NCORES = 8
TOK = 512            # owned tokens per core
EXT = TOK + 4        # fused ext window: tokens -2 .. TOK+2
G1E = TOK + 2        # conv1 ext output: tokens -1 .. TOK+1
KT = L // 128        # 24
KH = KT // 2         # 12
DT = D // 128        # 8
DA = 4               # d-tiles done k-major in phase A
NMEL = 100
NM1 = NMEL + 1       # mel lhsT cols: 96 rows | ones | 4 displaced rows
GS = 64              # group size (1024 / 16 groups)
GROUPS_ = 16

OFF_C1 = 0
OFF_C2 = OFF_C1 + NM1
OFF_ONES = OFF_C2 + NM1
OFF_TW = OFF_ONES + TOK      # 2.0s: ps_s lhsT (rstd = 2 * Dsqrt output)
SM_LEN = OFF_TW + NM1

LN_EPS = 1e-5
NWU = 72             # PE warmup matmuls: bridge the DMA ramp (~8us) + data
                     # wait (~15us) so phase A starts with HAM at 2.4 GHz

_PROGRAM = None          # cached (nc, input names)
LAST_RESULTS = None      # BassKernelResults of the most recent run (for test.py)


def _build_program():
    _ensure_concourse()
    from concourse import bacc, tile
    import concourse.mybir as mybir

    f32 = mybir.dt.float32
    f32r = mybir.dt.float32r
    bf16 = mybir.dt.bfloat16
    AF = mybir.ActivationFunctionType
    MUL = mybir.AluOpType.mult
    ADD = mybir.AluOpType.add

    nc = bacc.Bacc("TRN2", debug=False, target_bir_lowering=False)

    # DRAM layouts are partition-major so every DMA is contiguous.
    xt_d = nc.dram_tensor("xt", [KT, 128, EXT], bf16, kind="ExternalInput")
    wbig_d = nc.dram_tensor("wbig", [DT * 2, 128, KH, 128], bf16,
                            kind="ExternalInput")
    cw1_d = nc.dram_tensor("cw1", [128, DT, 3, 128], bf16, kind="ExternalInput")
    cw2_d = nc.dram_tensor("cw2", [128, DT, 3, 128], bf16, kind="ExternalInput")
    wgt_d = nc.dram_tensor("wgt", [128, DT, NM1], bf16, kind="ExternalInput")
    cb_d = nc.dram_tensor("cb", [128, 36], f32, kind="ExternalInput")
    sm_d = nc.dram_tensor("smalls", [1, SM_LEN], f32r, kind="ExternalInput")
    onec_d = nc.dram_tensor("onec", [128, 1], bf16, kind="ExternalInput")
    # host-computed halo columns: per d-tile, 4 fused halo cols + 2 g halo cols
    halo_d = nc.dram_tensor("halo", [128, DT, 6], bf16, kind="ExternalInput")
    mel_d = nc.dram_tensor("mel", [NMEL, TOK], f32, kind="ExternalOutput")

    with tile.TileContext(nc) as tc:
        with (
            tc.tile_pool(name="consts", bufs=1) as consts,
            tc.tile_pool(name="wpool", bufs=16) as wpool,
            tc.tile_pool(name="acts", bufs=1) as acts,
            tc.tile_pool(name="stats", bufs=1) as stats,
            tc.tile_pool(name="ps_ga", bufs=4, space="PSUM") as ps_ga,
            tc.tile_pool(name="ps_cv", bufs=2, space="PSUM") as ps_cv,
            tc.tile_pool(name="ps_sq", bufs=1, space="PSUM") as ps_sq_p,
            tc.tile_pool(name="ps_mel", bufs=1, space="PSUM") as ps_mel,
        ):
            wu_sb = consts.tile([128, 128], bf16, name="wu_sb")
            nc.vector.memset(wu_sb, 0.0)
            # ---- PE warmup while the first DMAs stream ----
            ps_wu = ps_cv.tile([128, 128], f32, name="ps_wu", tag="cv")
            for i in range(NWU):
                nc.tensor.matmul(
                    ps_wu, lhsT=wu_sb, rhs=wu_sb,
                    start=(i == 0), stop=(i == NWU - 1),
                )

            wbh = {}

            def load_wbh(i):
                t = wpool.tile([128, KH, 128], bf16, name=f"wbh{i}", tag="wb")
                nc.sync.dma_start(out=t, in_=wbig_d[i])
                wbh[i] = t

            xk = []

            def load_xk(k):
                t = consts.tile([128, EXT], bf16, name=f"xk{k}", tag=f"xk{k}")
                nc.sync.dma_start(out=t, in_=xt_d[k])
                xk.append(t)

            load_wbh(0)
            load_xk(0)
            load_wbh(2)
            load_xk(1)
            load_wbh(4)
            load_xk(2)
            load_wbh(6)
            load_xk(3)
            sm_sb = consts.tile([1, SM_LEN], f32r, name="sm_sb")
            nc.sync.dma_start(out=sm_sb, in_=sm_d[:])
            cb_sb = consts.tile([128, 36], f32, name="cb_sb")
            nc.sync.dma_start(out=cb_sb, in_=cb_d[:])
            ones_col = consts.tile([128, 1], bf16, name="ones_col")
            nc.sync.dma_start(out=ones_col, in_=onec_d[:])
            halo_sb = consts.tile([128, DT, 6], bf16, name="halo_sb")
            nc.sync.dma_start(out=halo_sb, in_=halo_d[:])
            ones_row = sm_sb[0:1, OFF_ONES : OFF_ONES + TOK]
            for k in range(4, 12):
                load_xk(k)
            load_wbh(1)
            load_wbh(3)
            for k in range(12, 16):
                load_xk(k)
            load_wbh(5)
            load_wbh(7)
            load_xk(16)
            load_xk(17)
            load_wbh(8)
            for k in range(18, 21):
                load_xk(k)
            load_wbh(9)
            for k in range(21, 24):
                load_xk(k)
            cw1_sb = consts.tile([128, DT, 3, 128], bf16, name="cw1_sb")
            cw2_sb = consts.tile([128, DT, 3, 128], bf16, name="cw2_sb")
            wgt_sb = consts.tile([128, DT, NM1], bf16, name="wgt_sb")
            nc.scalar.dma_start(out=cw1_sb, in_=cw1_d[:])
            nc.scalar.dma_start(out=cw2_sb, in_=cw2_d[:])
            nc.scalar.dma_start(out=wgt_sb, in_=wgt_d[:])
            for i in range(10, 16):
                load_wbh(i)

            fused = [None] * DT
            g = [None] * DT
            h2 = [None] * DT
            h2sq = [None] * DT
            ps_sq_ref = [None]
            ps_m_ref = [None]

            def fused_copy(d, ps):
                fu = acts.tile([128, EXT], bf16, name=f"fu{d}", tag=f"fu{d}")
                fused[d] = fu
                # bias add + f32->bf16 cast in one scalar op
                nc.scalar.add(out=fu[:, 2 : 2 + TOK], in_=ps,
                              add=cb_sb[:, 27 + d : 28 + d])
                nc.vector.tensor_copy(fu[:, 0:2], halo_sb[:, d, 0:2])
                nc.vector.tensor_copy(fu[:, EXT - 2 : EXT], halo_sb[:, d, 2:4])

            # ---- phase A: k-major big GEMM for d-tiles 0..3 ----
            psA = [ps_ga.tile([128, TOK], f32, name=f"psA{d}", tag="ga")
                   for d in range(DA)]
            for k in range(KT):
                for d in range(DA):
                    nc.tensor.matmul(
                        psA[d],
                        lhsT=wbh[2 * d + k // KH][:, k % KH, :],
                        rhs=xk[k][:, 2 : 2 + TOK],
                        start=(k == 0), stop=(k == KT - 1),
                    )
                    if k == KT - 1:
                        # drain each bank as soon as its group stops so the
                        # phase-B gemm can reuse it without a scalar-op stall
                        fused_copy(d, psA[d])
                if k < 12:
                    # filler matmuls: while the x stream still trickles in,
                    # keep the PE busy through data-wait gaps so HAM doesn't
                    # re-throttle the clock to 1.2 GHz (~50ns each when warm)
                    for _ in range(3):
                        nc.tensor.matmul(ps_wu, lhsT=wu_sb, rhs=wu_sb,
                                         start=True, stop=True)

            # ---- phase B/C building blocks ----
            def gemm(d):
                ps = ps_ga.tile([128, TOK], f32, name=f"psA{d}", tag="ga")
                for k in range(KT):
                    nc.tensor.matmul(
                        ps,
                        lhsT=wbh[2 * d + k // KH][:, k % KH, :],
                        rhs=xk[k][:, 2 : 2 + TOK],
                        start=(k == 0), stop=(k == KT - 1),
                    )
                fused_copy(d, ps)

            def conv1(d):
                # device computes g_ext cols [1, 513); cols 0 and 513 from host
                gd = acts.tile([128, G1E], bf16, name=f"g{d}", tag=f"g{d}")
                g[d] = gd
                ps = ps_cv.tile([128, TOK], f32, name=f"psB{d}", tag="cv")
                for tap in range(3):
                    nc.tensor.matmul(
                        ps, lhsT=cw1_sb[:, d, tap, :],
                        rhs=fused[d][:, 1 + tap : 1 + tap + TOK],
                        start=(tap == 0), stop=(tap == 2),
                    )
                # gelu(conv1 + b1) in one ACT-engine op (erf-based LUT)
                nc.scalar.activation(
                    out=gd[:, 1 : 1 + TOK], in_=ps, func=AF.Gelu,
                    bias=cb_sb[:, 19 + d : 20 + d], scale=1.0,
                )
                nc.vector.tensor_copy(gd[:, 0:1], halo_sb[:, d, 4:5])
                nc.vector.tensor_copy(gd[:, G1E - 1 : G1E], halo_sb[:, d, 5:6])

            def conv2(d):
                h2d = acts.tile([128, TOK], bf16, name=f"h2{d}", tag=f"h2{d}")
                h2sqd = acts.tile([128, TOK], bf16, name=f"h2sq{d}", tag="h2sq",
                                  bufs=2)
                h2[d] = h2d
                h2sq[d] = h2sqd
                ps = ps_cv.tile([128, TOK], f32, name=f"psC{d}", tag="cv")
                for tap in range(3):
                    nc.tensor.matmul(
                        ps, lhsT=cw2_sb[:, d, tap, :],
                        rhs=g[d][:, tap : tap + TOK],
                        start=(tap == 0), stop=(tap == 2),
                    )
                nc.scalar.add(out=h2d, in_=ps, add=cb_sb[:, 8 + d : 9 + d])
                # square on DVE (bf16 in/out, 2x rate) — scalar engine is the
                # phase-C pipeline bottleneck otherwise
                nc.vector.tensor_mul(h2sqd, h2d, h2d)

            def statmm(d):
                if d == 0:
                    ps_sq_ref[0] = ps_sq_p.tile([1, TOK], f32, name="ps_sq",
                                                tag="sq")
                    ps_m_ref[0] = ps_mel.tile([NM1, TOK], f32, name="ps_m",
                                              tag="mel")
                last = d == DT - 1
                nc.tensor.matmul(ps_sq_ref[0], lhsT=ones_col, rhs=h2sq[d][:],
                                 start=(d == 0), stop=last)
                # rows 0..95: Wg rows 0..95; row 96: channel sum (mean);
                # rows 97..100: Wg rows 96..99
                nc.tensor.matmul(ps_m_ref[0], lhsT=wgt_sb[:, d, :], rhs=h2[d][:],
                                 start=(d == 0), stop=last)

            # ---- phase B: d-major gemms 4..7 + pipelined conv/stats ----
            for _ in range(8):
                # bridge the wbh8/9 DMA wait at the A->B transition
                nc.tensor.matmul(ps_wu, lhsT=wu_sb, rhs=wu_sb,
                                 start=True, stop=True)
            gemm(4)
            conv1(0)
            gemm(5)
            conv1(1)
            conv2(0)
            gemm(6)
            conv1(2)
            conv2(1)
            statmm(0)
            gemm(7)
            conv1(3)
            conv2(2)
            statmm(1)
            # ---- phase C: remaining conv/stats ----
            for d in range(4, DT):
                conv1(d)
                conv2(d - 1)
                statmm(d - 2)
            conv2(DT - 1)
            statmm(DT - 2)
            statmm(DT - 1)

            # ---- stats on [1, TOK] ----
            # the mel lhsT's fold column is prescaled to 1/D (exact in bf16)
            # so ps_m row 96 IS the mean; the sq ones-column is prescaled
            # likewise so ps_sq is E[h^2].
            ps_sq = ps_sq_ref[0]
            ps_m = ps_m_ref[0]
            msq = stats.tile([1, TOK], f32, name="msq", tag="sv", bufs=3)
            nc.scalar.activation(msq, ps_m[96:97, :], AF.Square)
            var = stats.tile([1, TOK], f32, name="var", tag="sv", bufs=3)
            nc.vector.scalar_tensor_tensor(
                var, in0=msq, scalar=-1.0, in1=ps_sq, op0=MUL, op1=ADD,
            )
            sqv = stats.tile([1, TOK], f32, name="sqv", tag="sv", bufs=3)
            nc.scalar.activation(sqv, var, AF.Sqrt,
                                 bias=cb_sb[0:1, 18:19], scale=1.0)
            rstd32 = stats.tile([1, TOK], f32, name="rstd32")
            nc.vector.reciprocal_approx_fast(rstd32, sqv)
            rstd = stats.tile([1, TOK], f32r, name="rstd")
            nc.vector.tensor_copy(rstd, rstd32)
            negu = stats.tile([1, TOK], f32r, name="negu")
            nc.vector.scalar_tensor_tensor(
                negu, in0=ps_m[96:97, :], scalar=-1.0, in1=rstd32,
                op0=MUL, op1=MUL,
            )
            # drain the mel PSUM to SBUF while the correction matmuls run
            m_sb = stats.tile([NM1, TOK], f32, name="m_sb")
            nc.vector.tensor_copy(m_sb, ps_m)

            # ---- rank-1 corrections + output ----
            ps_r = ps_cv.tile([NM1, TOK], f32, name="ps_r", tag="cv")
            nc.tensor.matmul(
                ps_r, lhsT=sm_sb[0:1, OFF_C2 : OFF_C2 + NM1],
                rhs=ones_row, start=True, stop=False,
            )
            nc.tensor.matmul(
                ps_r, lhsT=sm_sb[0:1, OFF_C1 : OFF_C1 + NM1],
                rhs=negu[0:1, :], start=False, stop=True,
            )
            ps_s = ps_cv.tile([NM1, TOK], f32, name="ps_s", tag="cv")
            nc.tensor.matmul(
                ps_s, lhsT=sm_sb[0:1, OFF_ONES : OFF_ONES + NM1],
                rhs=rstd[0:1, :], start=True, stop=True,
            )
            out_sb = stats.tile([NM1, TOK], f32, name="out_sb")
            nc.vector.tensor_mul(out_sb, m_sb, ps_s)
            nc.vector.tensor_add(out_sb, out_sb, ps_r)
            nc.sync.dma_start(out=mel_d[0:96, :], in_=out_sb[0:96, :])
            nc.sync.dma_start(out=mel_d[96:NMEL, :], in_=out_sb[97:NM1, :])

    nc.compile()
    return nc


def _sigmoid64(x):
    return 1.0 / (1.0 + np.exp(-x.astype(np.float64)))


def _melperm(v):
    """Permute a [100]-vector into the 101-slot layout (slot 96 = 0)."""
    out = np.zeros(NM1, dtype=v.dtype)
    out[0:96] = v[0:96]
    out[97:NM1] = v[96:NMEL]
    return out


def host_prep(inputs):
    """Fold all data-independent computation; build per-core device inputs.

    Returns (shared, per_core) where shared is a dict of replicated arrays
    and per_core is a list of 8 dicts with the core-specific arrays.
    """
    import ml_dtypes

    f32 = np.float32
    bf16 = ml_dtypes.bfloat16
    W_in = np.asarray(inputs["W_in"], dtype=np.float64)
    Wd = np.asarray(inputs["Wd"], dtype=np.float64)
    bd = np.asarray(inputs["bd"], dtype=np.float64)
    P = np.asarray(inputs["P"], dtype=np.float64)
    smask = np.asarray(inputs["spectral_mask"], dtype=np.float64)
    b_in = np.asarray(inputs["b_in"], dtype=np.float64)

    w_proj = W_in @ Wd.T + bd[None, :]
    w_prime = P.T @ w_proj @ P
    masked_w = w_prime * _sigmoid64(smask)
    A = P @ masked_w.T @ P.T
    W_big64 = W_in.T @ A                                       # [L, D] f64
    b_big64 = b_in @ A                                         # [D] f64
    W_big = np.ascontiguousarray(W_big64, dtype=f32)

    # [2d+half, kp, k_in_half, dc] (partition-major, half k-slices)
    wbig_t = np.ascontiguousarray(
        W_big.reshape(2, KH, 128, DT, 128).transpose(3, 0, 2, 1, 4)
    ).reshape(DT * 2, 128, KH, 128).astype(bf16)

    def blockdiag(w):
        w = np.asarray(w, dtype=f32)  # [C, GS, 3]
        out = np.zeros((DT, 3, 128, 128), dtype=f32)
        for d in range(DT):
            for co in range(128):
                c = d * 128 + co
                blk = co // GS
                # out[d, tap, blk*GS + i, co] = w[c, i, tap]
                out[d, :, blk * GS : (blk + 1) * GS, co] = w[c].T
        return out

    cw1_t = np.ascontiguousarray(
        blockdiag(inputs["conv1_w"]).transpose(2, 0, 1, 3)).astype(bf16)
    cw2_t = np.ascontiguousarray(
        blockdiag(inputs["conv2_w"]).transpose(2, 0, 1, 3)).astype(bf16)

    Wmel = np.asarray(inputs["Wmel"], dtype=np.float64)
    ln_g = np.asarray(inputs["ln_g"], dtype=np.float64)
    ln_b = np.asarray(inputs["ln_b"], dtype=np.float64)
    bmel = np.asarray(inputs["bmel"], dtype=np.float64)
    Wg = (Wmel * ln_g[None, :]).astype(f32)                    # [NMEL, D]
    # lhsT col j: j<96 -> Wg row j; 96 -> ones (mean fold); 97..100 -> rows 96..99
    wgt_t = np.zeros((128, DT, NM1), dtype=f32)
    wgt_full = Wg.T.reshape(DT, 128, NMEL).transpose(1, 0, 2)  # [p, d, m]
    wgt_t[:, :, 0:96] = wgt_full[:, :, 0:96]
    wgt_t[:, :, 96] = 1.0 / D                    # mean fold (2^-10, exact)
    wgt_t[:, :, 97:NM1] = wgt_full[:, :, 96:NMEL]
    wgt_t = wgt_t.astype(bf16)
    c1 = _melperm((Wmel @ ln_g).astype(f32))
    c2 = _melperm((Wmel @ ln_b + bmel).astype(f32))

    cb_base = np.zeros((128, 36), dtype=f32)
    cb_base[:, 18] = LN_EPS
    b1_cols = np.asarray(inputs["conv1_b"], dtype=f32).reshape(DT, 128).T
    cb_base[:, 8:16] = np.asarray(inputs["conv2_b"], dtype=f32).reshape(DT, 128).T
    cb_base[:, 19:27] = b1_cols
    cb_base[:, 27:35] = b_big64.astype(f32).reshape(DT, 128).T

    llama = np.asarray(inputs["llama_embeddings"], dtype=f32).reshape(B * T, L)
    conv1_w_np = np.asarray(inputs["conv1_w"], dtype=np.float64)  # [D, GS, 3]
    conv1_b_np = np.asarray(inputs["conv1_b"], dtype=np.float64)
    gidx = np.arange(D) // GS

    import math
    _erf_vec = np.vectorize(math.erf)

    def _gelu64(x):
        return x * 0.5 * (1.0 + _erf_vec(x / math.sqrt(2.0)))

    shared = dict(wbig=wbig_t, cw1=cw1_t, cw2=cw2_t, wgt=wgt_t,
                  onec=np.full((128, 1), 1.0 / D, dtype=bf16))
    per_core = []
    for c in range(NCORES):
        b, h = divmod(c, 2)
        start = b * T + h * TOK
        ext_idx = np.arange(start - 2, start + TOK + 2)
        valid = (ext_idx >= b * T) & (ext_idx < (b + 1) * T)
        xext = np.zeros((EXT, L), dtype=f32)
        xext[valid] = llama[ext_idx[valid]]
        xt = np.ascontiguousarray(
            xext.T.reshape(KT, 128, EXT)
        ).astype(bf16)  # [k, p, t]

        # host-computed halo columns (exact fp32-grade)
        def fcol(u):
            gu = start + u
            if b * T <= gu < (b + 1) * T:
                return llama[gu].astype(np.float64) @ W_big64 + b_big64
            return np.zeros(D, dtype=np.float64)

        def conv1col(m3):
            # m3: [D, 3] inputs for taps 0..2 -> conv1 + bias, gelu
            in_g = m3.reshape(GROUPS_, GS, 3)[gidx]       # [D, GS, 3]
            out = np.einsum("cit,cit->c", conv1_w_np, in_g) + conv1_b_np
            return _gelu64(out)

        fm2, fm1, f0 = fcol(-2), fcol(-1), fcol(0)
        f510, f511 = fcol(510), fcol(511)
        f512, f513 = fcol(TOK), fcol(TOK + 1)
        if h == 1:
            g_left = conv1col(np.stack([fm2, fm1, f0], axis=1))
        else:
            g_left = np.zeros(D, dtype=np.float64)
        if h == 0:
            g_right = conv1col(np.stack([f511, f512, f513], axis=1))
        else:
            g_right = np.zeros(D, dtype=np.float64)
        halo = np.zeros((128, DT, 6), dtype=f32)
        for dd in range(DT):
            slc = slice(dd * 128, (dd + 1) * 128)
            halo[:, dd, 0] = fm2[slc]
            halo[:, dd, 1] = fm1[slc]
            halo[:, dd, 2] = f512[slc]
            halo[:, dd, 3] = f513[slc]
            halo[:, dd, 4] = g_left[slc]
            halo[:, dd, 5] = g_right[slc]

        sm = np.zeros((1, SM_LEN), dtype=f32)
        sm[0, OFF_C1 : OFF_C1 + NM1] = c1
        sm[0, OFF_C2 : OFF_C2 + NM1] = c2
        sm[0, OFF_ONES : OFF_ONES + TOK] = 1.0
        sm[0, OFF_TW : OFF_TW + NM1] = 2.0

        per_core.append(dict(xt=xt, smalls=sm, cb=cb_base,
                             halo=halo.astype(bf16)))
    return shared, per_core


def _ensure_axon_hooks():
    """If this image's antenv lacks axon_hooks (needed by bass_utils when
    BASS_TRACE is set under axon), register a functional ctypes-based hook so
    tracing degrades gracefully instead of crashing."""
    try:
        import antenv.axon_hooks  # noqa: F401
        return
    except ImportError:
        pass
    try:
        import contextlib
        import ctypes
        import types

        hook = None
        try:
            lib = ctypes.CDLL("/opt/axon/libaxon_pjrt.so")
            if hasattr(lib, "axon_start_nrt_profile"):
                lib.axon_start_nrt_profile.argtypes = [
                    ctypes.POINTER(ctypes.c_int64),
                    ctypes.c_size_t,
                ]
                lib.axon_start_nrt_profile.restype = ctypes.c_int64
                lib.axon_stop_nrt_profile.argtypes = [ctypes.c_char_p]
                lib.axon_stop_nrt_profile.restype = ctypes.c_int64

                @contextlib.contextmanager
                def hook(output_dir, device_ids):
                    import jax

                    jax.devices()
                    if device_ids:
                        ids = (ctypes.c_int64 * len(device_ids))(*device_ids)
                        rc = lib.axon_start_nrt_profile(ids, len(device_ids))
                    else:
                        rc = lib.axon_start_nrt_profile(None, 0)
                    if rc != 0:
                        raise RuntimeError(f"axon_start_nrt_profile rc={rc}")
                    try:
                        yield
                    finally:
                        lib.axon_stop_nrt_profile(str(output_dir).encode())
        except OSError:
            hook = None

        mod = types.ModuleType("antenv.axon_hooks")
        mod.get_axon_ntff_profile_hook = lambda: hook
        mod.set_axon_ntff_profile_hook = lambda h: None
        sys.modules["antenv.axon_hooks"] = mod
        import antenv

        antenv.axon_hooks = mod
    except Exception:
        pass


def kernel(**inputs):
    global _PROGRAM, LAST_RESULTS
    _ensure_concourse()
    _ensure_axon_hooks()
    from concourse import bass_utils

    if _PROGRAM is None:
        _PROGRAM = _build_program()
    nc = _PROGRAM

    shared, per_core = host_prep(inputs)
    in_maps = [{**shared, **pc} for pc in per_core]

    res = None
    last_exc = None
    for _attempt in range(3):
        try:
            res = bass_utils.run_bass_kernel_spmd(
                nc, in_maps, core_ids=list(range(NCORES))
            )
            break
        except Exception as exc:  # transient NRT device errors happen
            last_exc = exc
    if res is None:
        raise last_exc
    LAST_RESULTS = res

    out = np.zeros((B, NMEL, T), dtype=np.float32)
    for c in range(NCORES):
        b, h = divmod(c, 2)
        out[b, :, h * TOK : (h + 1) * TOK] = res.results[c]["mel"]
    return out


# revision 59
# speedup vs baseline: 1.0570x; 1.0422x over previous
"""Trainium2 Bass kernel for the CSMAdapter module.

Contract: kernel(**inputs) takes the FULL unsharded inputs (as produced by
the reference setup_inputs()) and returns the FULL output [4, 100, 1024].

Strategy
--------
All weight-only computation is folded on the host (it is data-independent):
    w_proj   = W_in @ Wd.T + bd
    w_prime  = P.T @ w_proj @ P
    masked_w = w_prime * sigmoid(spectral_mask)
    A        = P @ masked_w.T @ P.T          # fused = x @ A
    W_big    = W_in.T @ A                    # fused = llama @ W_big + b_in @ A
The final LayerNorm + mel projection algebra is folded into the mel GEMM:
    mel[m,t] = rstd[t]*(Wg @ h2)[m,t] - (mu[t]*rstd[t])*c1[m] + c2[m]
with Wg = Wmel * ln_g, c1 = Wmel @ ln_g, c2 = Wmel @ ln_b + bmel.
The channel mean is folded into the mel GEMM as an extra ones column at
lhsT position 96 (PSUM partition reads must be 32-aligned); the four mel
rows it displaces live at positions 97..100 and the output is DMA'd out
as two partition ranges.

Device (SPMD over 8 cores, data-parallel over the 4096 tokens, 512 each +
2-token conv halos), all heavy matmuls in bf16 (fp32 PSUM accumulation —
same 1 col/cycle PE rate as fp32r but half the HBM traffic):
  Phase A: big GEMM k-major over d-tiles 0-3 simultaneously (4 PSUM banks)
           so the PE stays saturated while the x k-tiles stream from HBM
           (one arriving k-tile unlocks 4 matmuls).
  Phase B: d-major big GEMM for d-tiles 4-7 (x now SBUF-resident),
           interleaved with conv1 -> gelu -> conv2 (block-diagonal per-tap
           matmuls, groups=16) and the mel/stats GEMMs for finished tiles.
  Tail:    LayerNorm stats chain on [1,512], rank-1 correction matmuls,
           combine, output DMA.
"""

import sys

import numpy as np


def _ensure_concourse():
    try:
        import concourse  # noqa: F401
    except ImportError:  # pragma: no cover
        for p in ("/opt/trn_rl_repo", "/root/.axon_site/_ro/trn_rl_repo"):
            if p not in sys.path:
                sys.path.insert(0, p)


# ---- static shapes ----
B, T, L, D = 4, 1024, 3072, 1024
NCORES = 8
TOK = 512            # owned tokens per core
EXT = TOK + 4        # fused ext window: tokens -2 .. TOK+2
G1E = TOK + 2        # conv1 ext output: tokens -1 .. TOK+1
KT = L // 128        # 24
KH = KT // 2         # 12
DT = D // 128        # 8
DA = 4               # d-tiles done k-major in phase A
NMEL = 100
NM1 = NMEL + 1       # mel lhsT cols: 96 rows | ones | 4 displaced rows
GS = 64              # group size (1024 / 16 groups)
GROUPS_ = 16

OFF_C1 = 0
OFF_C2 = OFF_C1 + NM1
OFF_ONES = OFF_C2 + NM1
OFF_TW = OFF_ONES + TOK      # 2.0s: ps_s lhsT (rstd = 2 * Dsqrt output)
SM_LEN = OFF_TW + NM1

LN_EPS = 1e-5
NWU = 72             # PE warmup matmuls: bridge the DMA ramp (~8us) + data
                     # wait (~15us) so phase A starts with HAM at 2.4 GHz

_PROGRAM = None          # cached (nc, input names)
LAST_RESULTS = None      # BassKernelResults of the most recent run (for test.py)


def _build_program():
    _ensure_concourse()
    from concourse import bacc, tile
    import concourse.mybir as mybir

    f32 = mybir.dt.float32
    f32r = mybir.dt.float32r
    bf16 = mybir.dt.bfloat16
    AF = mybir.ActivationFunctionType
    MUL = mybir.AluOpType.mult
    ADD = mybir.AluOpType.add

    nc = bacc.Bacc("TRN2", debug=False, target_bir_lowering=False)

    # DRAM layouts are partition-major so every DMA is contiguous.
    # x bundled as 4 groups of 6 k-tiles: 6KB-per-partition descriptor
    # lines (24 single-k transfers only manage 1KB lines and ~6x the
    # descriptor count, which starves the DMA engines mid-ramp)
    xt_d = nc.dram_tensor("xt", [4, 128, 6, EXT], bf16, kind="ExternalInput")
    wbig_d = nc.dram_tensor("wbig", [DT * 2, 128, KH, 128], bf16,
                            kind="ExternalInput")
    cw1_d = nc.dram_tensor("cw1", [128, DT, 3, 128], bf16, kind="ExternalInput")
    cw2_d = nc.dram_tensor("cw2", [128, DT, 3, 128], bf16, kind="ExternalInput")
    wgt_d = nc.dram_tensor("wgt", [128, DT, NM1], bf16, kind="ExternalInput")
    cb_d = nc.dram_tensor("cb", [128, 36], f32, kind="ExternalInput")
    sm_d = nc.dram_tensor("smalls", [1, SM_LEN], f32r, kind="ExternalInput")
    onec_d = nc.dram_tensor("onec", [128, 1], bf16, kind="ExternalInput")
    # host-computed halo columns: per d-tile, 4 fused halo cols + 2 g halo cols
    halo_d = nc.dram_tensor("halo", [128, DT, 6], bf16, kind="ExternalInput")
    mel_d = nc.dram_tensor("mel", [NMEL, TOK], f32, kind="ExternalOutput")

    with tile.TileContext(nc) as tc:
        with (
            tc.tile_pool(name="consts", bufs=1) as consts,
            tc.tile_pool(name="wpool", bufs=16) as wpool,
            tc.tile_pool(name="acts", bufs=1) as acts,
            tc.tile_pool(name="stats", bufs=1) as stats,
            tc.tile_pool(name="ps_ga", bufs=4, space="PSUM") as ps_ga,
            tc.tile_pool(name="ps_cv", bufs=2, space="PSUM") as ps_cv,
            tc.tile_pool(name="ps_sq", bufs=1, space="PSUM") as ps_sq_p,
            tc.tile_pool(name="ps_mel", bufs=1, space="PSUM") as ps_mel,
        ):
            wu_sb = consts.tile([128, 128], bf16, name="wu_sb")
            nc.vector.memset(wu_sb, 0.0)
            # ---- PE warmup while the first DMAs stream ----
            ps_wu = ps_cv.tile([128, 128], f32, name="ps_wu", tag="cv")
            for i in range(NWU):
                nc.tensor.matmul(
                    ps_wu, lhsT=wu_sb, rhs=wu_sb,
                    start=(i == 0), stop=(i == NWU - 1),
                )

            wbh = {}

            def load_wbh(i):
                t = wpool.tile([128, KH, 128], bf16, name=f"wbh{i}", tag="wb")
                nc.sync.dma_start(out=t, in_=wbig_d[i])
                wbh[i] = t

            xg = []

            def load_xg(j):
                t = consts.tile([128, 6, EXT], bf16, name=f"xg{j}", tag=f"xg{j}")
                nc.sync.dma_start(out=t, in_=xt_d[j])
                xg.append(t)

            def xk(k):
                return xg[k // 6][:, k % 6, :]

            load_wbh(0)
            load_xg(0)
            load_wbh(2)
            load_wbh(4)
            load_wbh(6)
            sm_sb = consts.tile([1, SM_LEN], f32r, name="sm_sb")
            nc.sync.dma_start(out=sm_sb, in_=sm_d[:])
            cb_sb = consts.tile([128, 36], f32, name="cb_sb")
            nc.sync.dma_start(out=cb_sb, in_=cb_d[:])
            ones_col = consts.tile([128, 1], bf16, name="ones_col")
            nc.sync.dma_start(out=ones_col, in_=onec_d[:])
            halo_sb = consts.tile([128, DT, 6], bf16, name="halo_sb")
            nc.sync.dma_start(out=halo_sb, in_=halo_d[:])
            ones_row = sm_sb[0:1, OFF_ONES : OFF_ONES + TOK]
            load_xg(1)
            load_wbh(1)
            load_wbh(3)
            load_xg(2)
            load_wbh(5)
            load_wbh(7)
            load_xg(3)
            load_wbh(8)
            load_wbh(9)
            cw1_sb = consts.tile([128, DT, 3, 128], bf16, name="cw1_sb")
            cw2_sb = consts.tile([128, DT, 3, 128], bf16, name="cw2_sb")
            wgt_sb = consts.tile([128, DT, NM1], bf16, name="wgt_sb")
            nc.scalar.dma_start(out=cw1_sb, in_=cw1_d[:])
            nc.scalar.dma_start(out=cw2_sb, in_=cw2_d[:])
            nc.scalar.dma_start(out=wgt_sb, in_=wgt_d[:])
            for i in range(10, 16):
                load_wbh(i)

            fused = [None] * DT
            g = [None] * DT
            h2 = [None] * DT
            h2sq = [None] * DT
            ps_sq_ref = [None]
            ps_m_ref = [None]

            def fused_copy(d, ps):
                fu = acts.tile([128, EXT], bf16, name=f"fu{d}", tag=f"fu{d}")
                fused[d] = fu
                # bias add + f32->bf16 cast in one scalar op
                nc.scalar.add(out=fu[:, 2 : 2 + TOK], in_=ps,
                              add=cb_sb[:, 27 + d : 28 + d])
                nc.vector.tensor_copy(fu[:, 0:2], halo_sb[:, d, 0:2])
                nc.vector.tensor_copy(fu[:, EXT - 2 : EXT], halo_sb[:, d, 2:4])

            # ---- phase A: k-major big GEMM for d-tiles 0..3 ----
            psA = [ps_ga.tile([128, TOK], f32, name=f"psA{d}", tag="ga")
                   for d in range(DA)]
            for k in range(KT):
                for d in range(DA):
                    nc.tensor.matmul(
                        psA[d],
                        lhsT=wbh[2 * d + k // KH][:, k % KH, :],
                        rhs=xk(k)[:, 2 : 2 + TOK],
                        start=(k == 0), stop=(k == KT - 1),
                    )
                    if k == KT - 1:
                        # drain each bank as soon as its group stops so the
                        # phase-B gemm can reuse it without a scalar-op stall
                        fused_copy(d, psA[d])
                if k < 12:
                    # filler matmuls: while the x stream still trickles in,
                    # keep the PE busy through data-wait gaps so HAM doesn't
                    # re-throttle the clock to 1.2 GHz (~50ns each when warm)
                    for _ in range(3):
                        nc.tensor.matmul(ps_wu, lhsT=wu_sb, rhs=wu_sb,
                                         start=True, stop=True)

            # ---- phase B/C building blocks ----
            def gemm(d):
                ps = ps_ga.tile([128, TOK], f32, name=f"psA{d}", tag="ga")
                for k in range(KT):
                    nc.tensor.matmul(
                        ps,
                        lhsT=wbh[2 * d + k // KH][:, k % KH, :],
                        rhs=xk(k)[:, 2 : 2 + TOK],
                        start=(k == 0), stop=(k == KT - 1),
                    )
                fused_copy(d, ps)

            def conv1(d):
                # device computes g_ext cols [1, 513); cols 0 and 513 from host
                gd = acts.tile([128, G1E], bf16, name=f"g{d}", tag=f"g{d}")
                g[d] = gd
                ps = ps_cv.tile([128, TOK], f32, name=f"psB{d}", tag="cv")
                for tap in range(3):
                    nc.tensor.matmul(
                        ps, lhsT=cw1_sb[:, d, tap, :],
                        rhs=fused[d][:, 1 + tap : 1 + tap + TOK],
                        start=(tap == 0), stop=(tap == 2),
                    )
                # gelu(conv1 + b1) in one ACT-engine op (erf-based LUT)
                nc.scalar.activation(
                    out=gd[:, 1 : 1 + TOK], in_=ps, func=AF.Gelu,
                    bias=cb_sb[:, 19 + d : 20 + d], scale=1.0,
                )
                nc.vector.tensor_copy(gd[:, 0:1], halo_sb[:, d, 4:5])
                nc.vector.tensor_copy(gd[:, G1E - 1 : G1E], halo_sb[:, d, 5:6])

            def conv2(d):
                h2d = acts.tile([128, TOK], bf16, name=f"h2{d}", tag=f"h2{d}")
                h2sqd = acts.tile([128, TOK], bf16, name=f"h2sq{d}", tag="h2sq",
                                  bufs=2)
                h2[d] = h2d
                h2sq[d] = h2sqd
                ps = ps_cv.tile([128, TOK], f32, name=f"psC{d}", tag="cv")
                for tap in range(3):
                    nc.tensor.matmul(
                        ps, lhsT=cw2_sb[:, d, tap, :],
                        rhs=g[d][:, tap : tap + TOK],
                        start=(tap == 0), stop=(tap == 2),
                    )
                nc.scalar.add(out=h2d, in_=ps, add=cb_sb[:, 8 + d : 9 + d])
                # square on DVE (bf16 in/out, 2x rate) — scalar engine is the
                # phase-C pipeline bottleneck otherwise
                nc.vector.tensor_mul(h2sqd, h2d, h2d)

            def statmm(d):
                if d == 0:
                    ps_sq_ref[0] = ps_sq_p.tile([1, TOK], f32, name="ps_sq",
                                                tag="sq")
                    ps_m_ref[0] = ps_mel.tile([NM1, TOK], f32, name="ps_m",
                                              tag="mel")
                last = d == DT - 1
                nc.tensor.matmul(ps_sq_ref[0], lhsT=ones_col, rhs=h2sq[d][:],
                                 start=(d == 0), stop=last)
                # rows 0..95: Wg rows 0..95; row 96: channel sum (mean);
                # rows 97..100: Wg rows 96..99
                nc.tensor.matmul(ps_m_ref[0], lhsT=wgt_sb[:, d, :], rhs=h2[d][:],
                                 start=(d == 0), stop=last)

            # ---- phase B: d-major gemms 4..7 + pipelined conv/stats ----
            for _ in range(8):
                # bridge the wbh8/9 DMA wait at the A->B transition
                nc.tensor.matmul(ps_wu, lhsT=wu_sb, rhs=wu_sb,
                                 start=True, stop=True)
            gemm(4)
            conv1(0)
            gemm(5)
            conv1(1)
            conv2(0)
            gemm(6)
            conv1(2)
            conv2(1)
            statmm(0)
            gemm(7)
            conv1(3)
            conv2(2)
            statmm(1)
            # ---- phase C: remaining conv/stats ----
            for d in range(4, DT):
                conv1(d)
                conv2(d - 1)
                statmm(d - 2)
            conv2(DT - 1)
            statmm(DT - 2)
            statmm(DT - 1)

            # ---- stats on [1, TOK] ----
            # the mel lhsT's fold column is prescaled to 1/D (exact in bf16)
            # so ps_m row 96 IS the mean; the sq ones-column is prescaled
            # likewise so ps_sq is E[h^2].
            ps_sq = ps_sq_ref[0]
            ps_m = ps_m_ref[0]
            msq = stats.tile([1, TOK], f32, name="msq", tag="sv", bufs=3)
            nc.scalar.activation(msq, ps_m[96:97, :], AF.Square)
            var = stats.tile([1, TOK], f32, name="var", tag="sv", bufs=3)
            nc.vector.scalar_tensor_tensor(
                var, in0=msq, scalar=-1.0, in1=ps_sq, op0=MUL, op1=ADD,
            )
            sqv = stats.tile([1, TOK], f32, name="sqv", tag="sv", bufs=3)
            nc.scalar.activation(sqv, var, AF.Sqrt,
                                 bias=cb_sb[0:1, 18:19], scale=1.0)
            rstd32 = stats.tile([1, TOK], f32, name="rstd32")
            nc.vector.reciprocal_approx_fast(rstd32, sqv)
            rstd = stats.tile([1, TOK], f32r, name="rstd")
            nc.vector.tensor_copy(rstd, rstd32)
            negu = stats.tile([1, TOK], f32r, name="negu")
            nc.vector.scalar_tensor_tensor(
                negu, in0=ps_m[96:97, :], scalar=-1.0, in1=rstd32,
                op0=MUL, op1=MUL,
            )
            # drain the mel PSUM to SBUF while the correction matmuls run
            m_sb = stats.tile([NM1, TOK], f32, name="m_sb")
            nc.vector.tensor_copy(m_sb, ps_m)

            # ---- rank-1 corrections + output ----
            ps_r = ps_cv.tile([NM1, TOK], f32, name="ps_r", tag="cv")
            nc.tensor.matmul(
                ps_r, lhsT=sm_sb[0:1, OFF_C2 : OFF_C2 + NM1],
                rhs=ones_row, start=True, stop=False,
            )
            nc.tensor.matmul(
                ps_r, lhsT=sm_sb[0:1, OFF_C1 : OFF_C1 + NM1],
                rhs=negu[0:1, :], start=False, stop=True,
            )
            ps_s = ps_cv.tile([NM1, TOK], f32, name="ps_s", tag="cv")
            nc.tensor.matmul(
                ps_s, lhsT=sm_sb[0:1, OFF_ONES : OFF_ONES + NM1],
                rhs=rstd[0:1, :], start=True, stop=True,
            )
            out_sb = stats.tile([NM1, TOK], f32, name="out_sb")
            nc.vector.tensor_mul(out_sb, m_sb, ps_s)
            nc.vector.tensor_add(out_sb, out_sb, ps_r)
            nc.sync.dma_start(out=mel_d[0:96, :], in_=out_sb[0:96, :])
            nc.sync.dma_start(out=mel_d[96:NMEL, :], in_=out_sb[97:NM1, :])

    nc.compile()
    return nc


def _sigmoid64(x):
    return 1.0 / (1.0 + np.exp(-x.astype(np.float64)))


def _melperm(v):
    """Permute a [100]-vector into the 101-slot layout (slot 96 = 0)."""
    out = np.zeros(NM1, dtype=v.dtype)
    out[0:96] = v[0:96]
    out[97:NM1] = v[96:NMEL]
    return out


def host_prep(inputs):
    """Fold all data-independent computation; build per-core device inputs.

    Returns (shared, per_core) where shared is a dict of replicated arrays
    and per_core is a list of 8 dicts with the core-specific arrays.
    """
    import ml_dtypes

    f32 = np.float32
    bf16 = ml_dtypes.bfloat16
    W_in = np.asarray(inputs["W_in"], dtype=np.float64)
    Wd = np.asarray(inputs["Wd"], dtype=np.float64)
    bd = np.asarray(inputs["bd"], dtype=np.float64)
    P = np.asarray(inputs["P"], dtype=np.float64)
    smask = np.asarray(inputs["spectral_mask"], dtype=np.float64)
    b_in = np.asarray(inputs["b_in"], dtype=np.float64)

    w_proj = W_in @ Wd.T + bd[None, :]
    w_prime = P.T @ w_proj @ P
    masked_w = w_prime * _sigmoid64(smask)
    A = P @ masked_w.T @ P.T
    W_big64 = W_in.T @ A                                       # [L, D] f64
    b_big64 = b_in @ A                                         # [D] f64
    W_big = np.ascontiguousarray(W_big64, dtype=f32)

    # [2d+half, kp, k_in_half, dc] (partition-major, half k-slices)
    wbig_t = np.ascontiguousarray(
        W_big.reshape(2, KH, 128, DT, 128).transpose(3, 0, 2, 1, 4)
    ).reshape(DT * 2, 128, KH, 128).astype(bf16)

    def blockdiag(w):
        w = np.asarray(w, dtype=f32)  # [C, GS, 3]
        out = np.zeros((DT, 3, 128, 128), dtype=f32)
        for d in range(DT):
            for co in range(128):
                c = d * 128 + co
                blk = co // GS
                # out[d, tap, blk*GS + i, co] = w[c, i, tap]
                out[d, :, blk * GS : (blk + 1) * GS, co] = w[c].T
        return out

    cw1_t = np.ascontiguousarray(
        blockdiag(inputs["conv1_w"]).transpose(2, 0, 1, 3)).astype(bf16)
    cw2_t = np.ascontiguousarray(
        blockdiag(inputs["conv2_w"]).transpose(2, 0, 1, 3)).astype(bf16)

    Wmel = np.asarray(inputs["Wmel"], dtype=np.float64)
    ln_g = np.asarray(inputs["ln_g"], dtype=np.float64)
    ln_b = np.asarray(inputs["ln_b"], dtype=np.float64)
    bmel = np.asarray(inputs["bmel"], dtype=np.float64)
    Wg = (Wmel * ln_g[None, :]).astype(f32)                    # [NMEL, D]
    # lhsT col j: j<96 -> Wg row j; 96 -> ones (mean fold); 97..100 -> rows 96..99
    wgt_t = np.zeros((128, DT, NM1), dtype=f32)
    wgt_full = Wg.T.reshape(DT, 128, NMEL).transpose(1, 0, 2)  # [p, d, m]
    wgt_t[:, :, 0:96] = wgt_full[:, :, 0:96]
    wgt_t[:, :, 96] = 1.0 / D                    # mean fold (2^-10, exact)
    wgt_t[:, :, 97:NM1] = wgt_full[:, :, 96:NMEL]
    wgt_t = wgt_t.astype(bf16)
    c1 = _melperm((Wmel @ ln_g).astype(f32))
    c2 = _melperm((Wmel @ ln_b + bmel).astype(f32))

    cb_base = np.zeros((128, 36), dtype=f32)
    cb_base[:, 18] = LN_EPS
    b1_cols = np.asarray(inputs["conv1_b"], dtype=f32).reshape(DT, 128).T
    cb_base[:, 8:16] = np.asarray(inputs["conv2_b"], dtype=f32).reshape(DT, 128).T
    cb_base[:, 19:27] = b1_cols
    cb_base[:, 27:35] = b_big64.astype(f32).reshape(DT, 128).T

    llama = np.asarray(inputs["llama_embeddings"], dtype=f32).reshape(B * T, L)
    conv1_w_np = np.asarray(inputs["conv1_w"], dtype=np.float64)  # [D, GS, 3]
    conv1_b_np = np.asarray(inputs["conv1_b"], dtype=np.float64)
    gidx = np.arange(D) // GS

    import math
    _erf_vec = np.vectorize(math.erf)

    def _gelu64(x):
        return x * 0.5 * (1.0 + _erf_vec(x / math.sqrt(2.0)))

    shared = dict(wbig=wbig_t, cw1=cw1_t, cw2=cw2_t, wgt=wgt_t,
                  onec=np.full((128, 1), 1.0 / D, dtype=bf16))
    per_core = []
    for c in range(NCORES):
        b, h = divmod(c, 2)
        start = b * T + h * TOK
        ext_idx = np.arange(start - 2, start + TOK + 2)
        valid = (ext_idx >= b * T) & (ext_idx < (b + 1) * T)
        xext = np.zeros((EXT, L), dtype=f32)
        xext[valid] = llama[ext_idx[valid]]
        xt = np.ascontiguousarray(
            xext.T.reshape(4, 6, 128, EXT).transpose(0, 2, 1, 3)
        ).astype(bf16)  # [g, p, kk, t]

        # host-computed halo columns (exact fp32-grade)
        def fcol(u):
            gu = start + u
            if b * T <= gu < (b + 1) * T:
                return llama[gu].astype(np.float64) @ W_big64 + b_big64
            return np.zeros(D, dtype=np.float64)

        def conv1col(m3):
            # m3: [D, 3] inputs for taps 0..2 -> conv1 + bias, gelu
            in_g = m3.reshape(GROUPS_, GS, 3)[gidx]       # [D, GS, 3]
            out = np.einsum("cit,cit->c", conv1_w_np, in_g) + conv1_b_np
            return _gelu64(out)

        fm2, fm1, f0 = fcol(-2), fcol(-1), fcol(0)
        f510, f511 = fcol(510), fcol(511)
        f512, f513 = fcol(TOK), fcol(TOK + 1)
        if h == 1:
            g_left = conv1col(np.stack([fm2, fm1, f0], axis=1))
        else:
            g_left = np.zeros(D, dtype=np.float64)
        if h == 0:
            g_right = conv1col(np.stack([f511, f512, f513], axis=1))
        else:
            g_right = np.zeros(D, dtype=np.float64)
        halo = np.zeros((128, DT, 6), dtype=f32)
        for dd in range(DT):
            slc = slice(dd * 128, (dd + 1) * 128)
            halo[:, dd, 0] = fm2[slc]
            halo[:, dd, 1] = fm1[slc]
            halo[:, dd, 2] = f512[slc]
            halo[:, dd, 3] = f513[slc]
            halo[:, dd, 4] = g_left[slc]
            halo[:, dd, 5] = g_right[slc]

        sm = np.zeros((1, SM_LEN), dtype=f32)
        sm[0, OFF_C1 : OFF_C1 + NM1] = c1
        sm[0, OFF_C2 : OFF_C2 + NM1] = c2
        sm[0, OFF_ONES : OFF_ONES + TOK] = 1.0
        sm[0, OFF_TW : OFF_TW + NM1] = 2.0

        per_core.append(dict(xt=xt, smalls=sm, cb=cb_base,
                             halo=halo.astype(bf16)))
    return shared, per_core


def _ensure_axon_hooks():
    """If this image's antenv lacks axon_hooks (needed by bass_utils when
    BASS_TRACE is set under axon), register a functional ctypes-based hook so
    tracing degrades gracefully instead of crashing."""
    try:
        import antenv.axon_hooks  # noqa: F401
        return
    except ImportError:
        pass
    try:
        import contextlib
        import ctypes
        import types

        hook = None
        try:
            lib = ctypes.CDLL("/opt/axon/libaxon_pjrt.so")
            if hasattr(lib, "axon_start_nrt_profile"):
                lib.axon_start_nrt_profile.argtypes = [
                    ctypes.POINTER(ctypes.c_int64),
                    ctypes.c_size_t,
                ]
                lib.axon_start_nrt_profile.restype = ctypes.c_int64
                lib.axon_stop_nrt_profile.argtypes = [ctypes.c_char_p]
                lib.axon_stop_nrt_profile.restype = ctypes.c_int64

                @contextlib.contextmanager
                def hook(output_dir, device_ids):
                    import jax

                    jax.devices()
                    if device_ids:
                        ids = (ctypes.c_int64 * len(device_ids))(*device_ids)
                        rc = lib.axon_start_nrt_profile(ids, len(device_ids))
                    else:
                        rc = lib.axon_start_nrt_profile(None, 0)
                    if rc != 0:
                        raise RuntimeError(f"axon_start_nrt_profile rc={rc}")
                    try:
                        yield
                    finally:
                        lib.axon_stop_nrt_profile(str(output_dir).encode())
        except OSError:
            hook = None

        mod = types.ModuleType("antenv.axon_hooks")
        mod.get_axon_ntff_profile_hook = lambda: hook
        mod.set_axon_ntff_profile_hook = lambda h: None
        sys.modules["antenv.axon_hooks"] = mod
        import antenv

        antenv.axon_hooks = mod
    except Exception:
        pass


def kernel(**inputs):
    global _PROGRAM, LAST_RESULTS
    _ensure_concourse()
    _ensure_axon_hooks()
    from concourse import bass_utils

    if _PROGRAM is None:
        _PROGRAM = _build_program()
    nc = _PROGRAM

    shared, per_core = host_prep(inputs)
    in_maps = [{**shared, **pc} for pc in per_core]

    res = None
    last_exc = None
    for _attempt in range(3):
        try:
            res = bass_utils.run_bass_kernel_spmd(
                nc, in_maps, core_ids=list(range(NCORES))
            )
            break
        except Exception as exc:  # transient NRT device errors happen
            last_exc = exc
    if res is None:
        raise last_exc
    LAST_RESULTS = res

    out = np.zeros((B, NMEL, T), dtype=np.float32)
    for c in range(NCORES):
        b, h = divmod(c, 2)
        out[b, :, h * TOK : (h + 1) * TOK] = res.results[c]["mel"]
    return out
